# revision 19
# baseline (speedup 1.0000x reference)
"""Trainium2 Bass kernel: 3-level db4 DWT front-end (analysis + per-band
single-band reconstructions).

Input  x : [16, 128, 8192] float32
Output   : [4, 16, 128, 8192] float32  (bands: approx, d3, d2, d1)

Sharding: depthwise per-(batch, channel) row -> flatten to 2048 independent
rows of length 8192; 256 rows per NeuronCore (8 cores), two [128, *]
partition tiles per core. No cross-core communication.

Compute layout (per 128-row tile):
  - analysis convs (stride 2, reflect pad 7, 8 taps): DVE fused MACs
    (scalar_tensor_tensor) with stride-2 reads; a2 chain on GPSIMD as
    mul+add pairs; first taps on ScalarE.
  - synthesis conv_transpose stages (two 4-tap phases, crop 7): TensorE
    diagonal matmuls in float32r accumulating in PSUM (4 taps per 512-col
    chunk), evacuated by ScalarE copies that also interleave (stride-2
    destination). Synthesis inputs are float32r-rounded tiles, as the
    fp32r matmul path requires.
"""

import numpy as np

import concourse.bass as bass
import concourse.tile as tile
from concourse import bacc, mybir
from concourse.bass_utils import run_bass_kernel_spmd

F32 = mybir.dt.float32
F32R = mybir.dt.float32r
MULT = mybir.AluOpType.mult
ADD = mybir.AluOpType.add
EQ = mybir.AluOpType.is_equal

DEC_LO = np.array([-0.0105974018, 0.0328830117, 0.0308413818, -0.1870348117,
                   -0.0279837694, 0.6308807679, 0.7148465706, 0.2303778133], np.float32)
DEC_HI = np.array([-0.2303778133, 0.7148465706, -0.6308807679, -0.0279837694,
                   0.1870348117, 0.0308413818, -0.0328830117, -0.0105974018], np.float32)
REC_LO = DEC_LO[::-1].copy()
REC_HI = DEC_HI[::-1].copy()

L0, L1, L2, L3 = 8192, 4100, 2054, 1031
N_CORES = 8
ROWS_PER_CORE = 256
TILES_PER_CORE = 2

V, P, S, PE = "vector", "gpsimd", "scalar", "pe"

# Engine per chain. Analysis chains: V (DVE STT) or P (GPSIMD mul+add).
# Synthesis stage-phases: PE (f32r diag matmul) or V.
ASSIGN = {
    "d1": V, "a1": V, "d2": V, "a2": P, "d3": V, "a3": V,
    "b3e": V, "b3o": V,
    "b2s1e": PE, "b2s1o": PE, "b2s2e": PE, "b2s2o": PE,
    "b1s1e": PE, "b1s1o": PE, "b1s2e": PE, "b1s2o": PE, "b1s3e": PE, "b1s3o": PE,
    "b0s1e": PE, "b0s1o": PE, "b0s2e": PE, "b0s2o": PE, "b0s3e": PE, "b0s3o": PE,
}
PSUM_CHUNK = 512


class Ctx:
    """Holds nc/pools/constant diag tiles during build."""

    def __init__(self, nc, pool, obpool, pspool, assign):
        self.nc = nc
        self.pool = pool
        self.obpool = obpool
        self.pspool = pspool
        self.assign = assign
        self.diag = {}  # float -> f32r diag tile

    def any_pe(self):
        return any(v == PE for v in self.assign.values())

    def build_consts(self):
        nc = self.nc
        ones = self.pool.tile([128, 128], F32, tag="ones")
        nc.vector.memset(ones[:], 1.0)
        ident = self.pool.tile([128, 128], F32, tag="ident")
        nc.gpsimd.affine_select(ident[:], ones[:], [[1, 128]], EQ, 0.0,
                                base=0, channel_multiplier=-1)
        vals = list(DEC_LO) + list(DEC_HI)
        for i, w in enumerate(vals):
            d = self.pool.tile([128, 128], F32R, tag=f"diag{i}")
            nc.vector.tensor_scalar_mul(d[:], ident[:], float(w))
            self.diag[float(np.float32(w))] = d


def _interleave(*op_lists):
    """Round-robin execute emission thunks so consecutive same-engine ops
    come from independent chains (hides the DVE pipe drain between the
    dependent in-place accumulation steps of a single chain)."""
    n = max(len(l) for l in op_lists)
    for i in range(n):
        for l in op_lists:
            if i < len(l):
                l[i]()


def _ana_thunks(ctx, xp, out, w, L):
    nc = ctx.nc
    No = L // 2 + 4
    ops = []
    for k in range(8):
        src = xp[:, k:k + 2 * No - 1:2]
        if k == 0:
            ops.append(lambda o=out, s=src, v=float(w[0]): nc.scalar.mul(o, s, v))
        else:
            ops.append(lambda o=out, s=src, v=float(w[k]):
                       nc.vector.scalar_tensor_tensor(o, s, v, o, MULT, ADD))
    return ops


def _emit_ana(ctx, xp, out, w, L, eng):
    """out[:, i] = sum_k w[k] * xp[:, 2i+k]  (No = L//2 + 4 outputs).
    eng=V: ACT first tap + DVE STT. eng=P: GPSIMD tensor_scalar muls +
    tensor_tensor adds via a scratch tile."""
    nc = ctx.nc
    No = L // 2 + 4
    if eng == V:
        _interleave(_ana_thunks(ctx, xp, out, w, L))
    elif eng == P:
        tmp = ctx.pool.tile([128, No], F32, tag="ptmp")
        for k in range(8):
            src = xp[:, k:k + 2 * No - 1:2]
            if k == 0:
                nc.gpsimd.tensor_scalar_mul(out, src, float(w[0]))
            else:
                nc.gpsimd.tensor_scalar_mul(tmp[:, :No], src, float(w[k]))
                nc.gpsimd.tensor_tensor(out, out, tmp[:, :No], ADD)
    else:
        raise ValueError(eng)


def _synth_phase_taps(w, phase):
    """(x_offset, weight) pairs for one conv_transpose phase after crop 7."""
    if phase == 0:  # dest[:, 2i] = sum_b w[7-2b] x[i+b]
        return [(b, w[7 - 2 * b]) for b in range(4)]
    return [(c, w[8 - 2 * c]) for c in range(1, 5)]  # dest[:, 2i+1]


def _synth_v_thunks(ctx, x, dphase, taps, H):
    nc = ctx.nc
    ops = []
    for i, (off, wv) in enumerate(taps):
        src = x[:, off:off + H]
        if i == 0:
            ops.append(lambda o=dphase, s=src, v=float(wv): nc.scalar.mul(o, s, v))
        else:
            ops.append(lambda o=dphase, s=src, v=float(wv):
                       nc.vector.scalar_tensor_tensor(o, s, v, o, MULT, ADD))
    return ops


def _emit_synth_phase(ctx, x, dest, w, T, phase, eng):
    """One phase (even/odd) of a synthesis stage: H = T//2 outputs written
    to dest[:, phase::2]."""
    nc = ctx.nc
    H = T // 2
    taps = _synth_phase_taps(w, phase)
    dphase = dest[:, phase:T:2]
    if eng == V:
        _interleave(_synth_v_thunks(ctx, x, dphase, taps, H))
    elif eng == PE:
        He = H - (H % 2)  # fp32r matmul PSUM dest requires an even column count
        for c0 in range(0, He, PSUM_CHUNK):
            n = min(PSUM_CHUNK, He - c0)
            ps = ctx.pspool.tile([128, PSUM_CHUNK], F32, tag="ps")
            for i, (off, wv) in enumerate(taps):
                rhs = x[:, c0 + off:c0 + off + n]
                nc.tensor.matmul(ps[:, :n], ctx.diag[float(np.float32(wv))][:],
                                 rhs, start=(i == 0), stop=(i == 3))
            s0 = phase + 2 * c0
            nc.scalar.copy(dest[:, s0:s0 + 2 * n - 1:2], ps[:, :n])
        if He < H:  # odd tail column on DVE
            c0 = He
            dcol = dest[:, phase + 2 * c0:phase + 2 * c0 + 1]
            for i, (off, wv) in enumerate(taps):
                src = x[:, c0 + off:c0 + off + 1]
                if i == 0:
                    nc.scalar.mul(dcol, src, float(wv))
                else:
                    nc.vector.scalar_tensor_tensor(dcol, src, float(wv), dcol,
                                                   MULT, ADD)
    else:
        raise ValueError(eng)


def _emit_synth(ctx, x, dest, w, T, key):
    _emit_synth_phase(ctx, x, dest, w, T, 0, ctx.assign[key + "e"])
    _emit_synth_phase(ctx, x, dest, w, T, 1, ctx.assign[key + "o"])


def _emit_reflect(ctx, xp, L):
    nc = ctx.nc
    nc.vector.tensor_copy(xp[:, 0:7], xp[:, 14:7:-1])
    nc.vector.tensor_copy(xp[:, 7 + L:14 + L], xp[:, L + 5:L - 2:-1])


def build_nc(assign=None):
    a = dict(ASSIGN)
    if assign:
        a.update(assign)
    nc = bacc.Bacc("TRN2", target_bir_lowering=False, debug=False,
                   num_devices=N_CORES)
    x_ap = nc.dram_tensor("x", [ROWS_PER_CORE, L0], F32, kind="ExternalInput").ap()
    y_ap = nc.dram_tensor("y", [4, ROWS_PER_CORE, L0], F32, kind="ExternalOutput").ap()

    with tile.TileContext(nc) as tc:
        with tc.tile_pool(name="bufs", bufs=1) as pool, \
             tc.tile_pool(name="ob", bufs=2) as obpool, \
             tc.tile_pool(name="ps", bufs=8, space="PSUM") as pspool:
            ctx = Ctx(nc, pool, obpool, pspool, a)
            ctx.build_consts()
            # Per-tile overrides: tile0 runs band3 on PE (d1 is ready early,
            # so PE starts ~90us sooner); the last tile runs band0's final
            # stage on DVE (fills DVE's tail idle while PE finishes band1).
            tile_over = [
                {"b3e": PE, "b3o": PE},
                {"b0s3e": V, "b0s3o": V},
            ]
            base_assign = dict(a)

            for t in range(TILES_PER_CORE):
                a = dict(base_assign)
                a.update(tile_over[t % len(tile_over)])
                ctx.assign = a

                # dtype of synthesis-stage inputs: f32r when consumed by PE
                def syn_dt(key):
                    return F32R if (a[key + "e"] == PE or a[key + "o"] == PE) else F32

                rows = slice(t * 128, (t + 1) * 128)

                xp0 = pool.tile([128, L0 + 14], F32, tag="xp0")
                nc.sync.dma_start(xp0[:, 7:7 + L0], x_ap[rows, :])
                _emit_reflect(ctx, xp0, L0)

                d1 = pool.tile([128, L1], syn_dt("b3"), tag="d1")
                a1p = pool.tile([128, L1 + 14], F32, tag="a1p")
                if a["b3e"] == PE:
                    # d1 gates PE's first work: emit it solo, split into two
                    # half-chains so band3's first PE chunks only wait for
                    # the first half, then a1.
                    Hh = L1 // 2  # 2050: out cols [0, Hh) read xp0[0:2*Hh+6]
                    for k in range(8):
                        src = xp0[:, k:k + 2 * Hh - 1:2]
                        dst = d1[:, 0:Hh]
                        if k == 0:
                            nc.scalar.mul(dst, src, float(DEC_HI[0]))
                        else:
                            nc.vector.scalar_tensor_tensor(
                                dst, src, float(DEC_HI[k]), dst, MULT, ADD)
                    n2 = L1 - Hh
                    for k in range(8):
                        src = xp0[:, 2 * Hh + k:2 * Hh + k + 2 * n2 - 1:2]
                        dst = d1[:, Hh:L1]
                        if k == 0:
                            nc.scalar.mul(dst, src, float(DEC_HI[0]))
                        else:
                            nc.vector.scalar_tensor_tensor(
                                dst, src, float(DEC_HI[k]), dst, MULT, ADD)
                    _emit_ana(ctx, xp0, a1p[:, 7:7 + L1], DEC_LO, L0, a["a1"])
                elif a["d1"] == V and a["a1"] == V:
                    _interleave(_ana_thunks(ctx, xp0, d1[:], DEC_HI, L0),
                                _ana_thunks(ctx, xp0, a1p[:, 7:7 + L1], DEC_LO, L0))
                else:
                    _emit_ana(ctx, xp0, d1[:], DEC_HI, L0, a["d1"])
                    _emit_ana(ctx, xp0, a1p[:, 7:7 + L1], DEC_LO, L0, a["a1"])
                _emit_reflect(ctx, a1p, L1)

                def emit_b3(d1=d1, rows=rows):
                    ob3 = obpool.tile([128, L0], F32, tag="ob")
                    if a["b3e"] == V and a["b3o"] == V:
                        _interleave(
                            _synth_v_thunks(ctx, d1, ob3[:, 0:L0:2],
                                            _synth_phase_taps(REC_HI, 0), L0 // 2),
                            _synth_v_thunks(ctx, d1, ob3[:, 1:L0:2],
                                            _synth_phase_taps(REC_HI, 1), L0 // 2))
                    else:
                        _emit_synth(ctx, d1, ob3, REC_HI, L0, "b3")
                    nc.sync.dma_start(y_ap[3, rows, :], ob3[:])

                if a["b3e"] == PE:
                    emit_b3()  # PE path: start it as early as possible

                d2 = pool.tile([128, L2], syn_dt("b2s1"), tag="d2")
                _emit_ana(ctx, a1p, d2[:], DEC_HI, L1, a["d2"])
                a2p = pool.tile([128, L2 + 14], F32, tag="a2p")
                _emit_ana(ctx, a1p, a2p[:, 7:7 + L2], DEC_LO, L1, a["a2"])
                _emit_reflect(ctx, a2p, L2)

                v_ = pool.tile([128, L1], syn_dt("b2s2"), tag="v")
                _emit_synth(ctx, d2, v_, REC_HI, L1, "b2s1")
                ob2 = obpool.tile([128, L0], F32, tag="ob")
                _emit_synth(ctx, v_, ob2, REC_LO, L0, "b2s2")
                nc.sync.dma_start(y_ap[2, rows, :], ob2[:])

                d3 = pool.tile([128, L3], syn_dt("b1s1"), tag="d3")
                a3 = pool.tile([128, L3], syn_dt("b0s1"), tag="a3")
                if a["d3"] == V and a["a3"] == V:
                    _interleave(_ana_thunks(ctx, a2p, d3[:], DEC_HI, L2),
                                _ana_thunks(ctx, a2p, a3[:], DEC_LO, L2))
                else:
                    _emit_ana(ctx, a2p, d3[:], DEC_HI, L2, a["d3"])
                    _emit_ana(ctx, a2p, a3[:], DEC_LO, L2, a["a3"])

                if a["b3e"] != PE:
                    # DVE band3: emit after all analysis so the PE-gating
                    # deep-level inputs (d2/d3/a3) are produced first.
                    emit_b3()

                u_ = pool.tile([128, L2], syn_dt("b1s2"), tag="u")
                _emit_synth(ctx, d3, u_, REC_HI, L2, "b1s1")
                v_ = pool.tile([128, L1], syn_dt("b1s3"), tag="v")
                _emit_synth(ctx, u_, v_, REC_LO, L1, "b1s2")
                ob1 = obpool.tile([128, L0], F32, tag="ob")
                _emit_synth(ctx, v_, ob1, REC_LO, L0, "b1s3")
                nc.sync.dma_start(y_ap[1, rows, :], ob1[:])

                u_ = pool.tile([128, L2], syn_dt("b0s2"), tag="u")
                _emit_synth(ctx, a3, u_, REC_LO, L2, "b0s1")
                v_ = pool.tile([128, L1], syn_dt("b0s3"), tag="v")
                _emit_synth(ctx, u_, v_, REC_LO, L1, "b0s2")
                ob0 = obpool.tile([128, L0], F32, tag="ob")
                _emit_synth(ctx, v_, ob0, REC_LO, L0, "b0s3")
                nc.sync.dma_start(y_ap[0, rows, :], ob0[:])

    nc.compile()
    return nc


_NC = None


def _get_nc():
    global _NC
    if _NC is None:
        _NC = build_nc()
    return _NC


def shard_inputs(x):
    rows = np.ascontiguousarray(x.reshape(-1, L0))
    return [{"x": rows[c * ROWS_PER_CORE:(c + 1) * ROWS_PER_CORE]}
            for c in range(N_CORES)]


def unshard_outputs(results):
    out = np.empty((4, N_CORES * ROWS_PER_CORE, L0), np.float32)
    for c, r in enumerate(results):
        out[:, c * ROWS_PER_CORE:(c + 1) * ROWS_PER_CORE, :] = r["y"]
    return out.reshape(4, 16, 128, L0)


def kernel(x):
    x = np.asarray(x, np.float32)
    assert x.shape == (16, 128, L0), x.shape
    nc = _get_nc()
    res = run_bass_kernel_spmd(nc, shard_inputs(x), core_ids=list(range(N_CORES)))
    return unshard_outputs(res.results)


# revision 20
# speedup vs baseline: 1.2543x; 1.2543x over previous
"""Trainium2 Bass kernel: 3-level db4 DWT front-end (analysis + per-band
single-band reconstructions).

Input  x : [16, 128, 8192] float32
Output   : [4, 16, 128, 8192] float32  (bands: approx, d3, d2, d1)

Sharding: depthwise per-(batch, channel) row -> flatten to 2048 independent
rows of length 8192; 256 rows per NeuronCore (8 cores), two [128, *]
partition tiles per core. No cross-core communication.

Compute layout (per 128-row tile):
  - analysis convs (stride 2, reflect pad 7, 8 taps): DVE fused MACs
    (scalar_tensor_tensor) with stride-2 reads; a2 chain on GPSIMD as
    mul+add pairs; first taps on ScalarE.
  - synthesis conv_transpose stages (two 4-tap phases, crop 7): TensorE
    diagonal matmuls in float32r accumulating in PSUM (4 taps per 512-col
    chunk), evacuated by ScalarE copies that also interleave (stride-2
    destination). Synthesis inputs are float32r-rounded tiles, as the
    fp32r matmul path requires.
"""

import numpy as np

import concourse.bass as bass
import concourse.tile as tile
from concourse import bacc, mybir
from concourse.bass_utils import run_bass_kernel_spmd

F32 = mybir.dt.float32
F32R = mybir.dt.float32r
MULT = mybir.AluOpType.mult
ADD = mybir.AluOpType.add
EQ = mybir.AluOpType.is_equal

DEC_LO = np.array([-0.0105974018, 0.0328830117, 0.0308413818, -0.1870348117,
                   -0.0279837694, 0.6308807679, 0.7148465706, 0.2303778133], np.float32)
DEC_HI = np.array([-0.2303778133, 0.7148465706, -0.6308807679, -0.0279837694,
                   0.1870348117, 0.0308413818, -0.0328830117, -0.0105974018], np.float32)
REC_LO = DEC_LO[::-1].copy()
REC_HI = DEC_HI[::-1].copy()

L0, L1, L2, L3 = 8192, 4100, 2054, 1031
N_CORES = 8
ROWS_PER_CORE = 256
TILES_PER_CORE = 2

V, P, S, PE = "vector", "gpsimd", "scalar", "pe"

# Engine per chain. Analysis chains: V (DVE STT) or P (GPSIMD mul+add).
# Synthesis stage-phases: PE (f32r diag matmul) or V.
ASSIGN = {
    "d1": V, "a1": V, "d2": V, "a2": P, "d3": V, "a3": V,
    "b3e": V, "b3o": V,
    "b2s1e": PE, "b2s1o": PE, "b2s2e": PE, "b2s2o": PE,
    "b1s1e": PE, "b1s1o": PE, "b1s2e": PE, "b1s2o": PE, "b1s3e": PE, "b1s3o": PE,
    "b0s1e": PE, "b0s1o": PE, "b0s2e": PE, "b0s2o": PE, "b0s3e": PE, "b0s3o": PE,
}
PSUM_CHUNK = 512


class Ctx:
    """Holds nc/pools/constant diag tiles during build."""

    def __init__(self, nc, pool, obpool, pspool, assign):
        self.nc = nc
        self.pool = pool
        self.obpool = obpool
        self.pspool = pspool
        self.assign = assign
        self.diag = {}  # float -> f32r diag tile

    def any_pe(self):
        return any(v == PE for v in self.assign.values())

    def build_consts(self):
        nc = self.nc
        ones = self.pool.tile([128, 128], F32, tag="ones")
        nc.vector.memset(ones[:], 1.0)
        ident = self.pool.tile([128, 128], F32, tag="ident")
        nc.gpsimd.affine_select(ident[:], ones[:], [[1, 128]], EQ, 0.0,
                                base=0, channel_multiplier=-1)
        vals = list(DEC_LO) + list(DEC_HI)
        for i, w in enumerate(vals):
            d = self.pool.tile([128, 128], F32R, tag=f"diag{i}")
            nc.vector.tensor_scalar_mul(d[:], ident[:], float(w))
            self.diag[float(np.float32(w))] = d


def _interleave(*op_lists):
    """Round-robin execute emission thunks so consecutive same-engine ops
    come from independent chains (hides the DVE pipe drain between the
    dependent in-place accumulation steps of a single chain)."""
    n = max(len(l) for l in op_lists)
    for i in range(n):
        for l in op_lists:
            if i < len(l):
                l[i]()


def _ana_thunks(ctx, xp, out, w, L):
    nc = ctx.nc
    No = L // 2 + 4
    ops = []
    for k in range(8):
        src = xp[:, k:k + 2 * No - 1:2]
        if k == 0:
            ops.append(lambda o=out, s=src, v=float(w[0]): nc.scalar.mul(o, s, v))
        else:
            ops.append(lambda o=out, s=src, v=float(w[k]):
                       nc.vector.scalar_tensor_tensor(o, s, v, o, MULT, ADD))
    return ops


def _emit_ana(ctx, xp, out, w, L, eng):
    """out[:, i] = sum_k w[k] * xp[:, 2i+k]  (No = L//2 + 4 outputs).
    eng=V: ACT first tap + DVE STT. eng=P: GPSIMD tensor_scalar muls +
    tensor_tensor adds via a scratch tile."""
    nc = ctx.nc
    No = L // 2 + 4
    if eng == V:
        _interleave(_ana_thunks(ctx, xp, out, w, L))
    elif eng == P:
        tmp = ctx.pool.tile([128, No], F32, tag="ptmp")
        for k in range(8):
            src = xp[:, k:k + 2 * No - 1:2]
            if k == 0:
                nc.gpsimd.tensor_scalar_mul(out, src, float(w[0]))
            else:
                nc.gpsimd.tensor_scalar_mul(tmp[:, :No], src, float(w[k]))
                nc.gpsimd.tensor_tensor(out, out, tmp[:, :No], ADD)
    else:
        raise ValueError(eng)


def _synth_phase_taps(w, phase):
    """(x_offset, weight) pairs for one conv_transpose phase after crop 7."""
    if phase == 0:  # dest[:, 2i] = sum_b w[7-2b] x[i+b]
        return [(b, w[7 - 2 * b]) for b in range(4)]
    return [(c, w[8 - 2 * c]) for c in range(1, 5)]  # dest[:, 2i+1]


def _synth_v_thunks(ctx, x, dphase, taps, H):
    nc = ctx.nc
    ops = []
    for i, (off, wv) in enumerate(taps):
        src = x[:, off:off + H]
        if i == 0:
            ops.append(lambda o=dphase, s=src, v=float(wv): nc.scalar.mul(o, s, v))
        else:
            ops.append(lambda o=dphase, s=src, v=float(wv):
                       nc.vector.scalar_tensor_tensor(o, s, v, o, MULT, ADD))
    return ops


def _emit_synth_phase(ctx, x, dest, w, T, phase, eng):
    """One phase (even/odd) of a synthesis stage: H = T//2 outputs written
    to dest[:, phase::2]."""
    nc = ctx.nc
    H = T // 2
    taps = _synth_phase_taps(w, phase)
    dphase = dest[:, phase:T:2]
    if eng == V:
        _interleave(_synth_v_thunks(ctx, x, dphase, taps, H))
    elif eng == PE:
        He = H - (H % 2)  # fp32r matmul PSUM dest requires an even column count
        for c0 in range(0, He, PSUM_CHUNK):
            n = min(PSUM_CHUNK, He - c0)
            ps = ctx.pspool.tile([128, PSUM_CHUNK], F32, tag="ps")
            for i, (off, wv) in enumerate(taps):
                rhs = x[:, c0 + off:c0 + off + n]
                nc.tensor.matmul(ps[:, :n], ctx.diag[float(np.float32(wv))][:],
                                 rhs, start=(i == 0), stop=(i == 3))
            s0 = phase + 2 * c0
            nc.scalar.copy(dest[:, s0:s0 + 2 * n - 1:2], ps[:, :n])
        if He < H:  # odd tail column on DVE
            c0 = He
            dcol = dest[:, phase + 2 * c0:phase + 2 * c0 + 1]
            for i, (off, wv) in enumerate(taps):
                src = x[:, c0 + off:c0 + off + 1]
                if i == 0:
                    nc.scalar.mul(dcol, src, float(wv))
                else:
                    nc.vector.scalar_tensor_tensor(dcol, src, float(wv), dcol,
                                                   MULT, ADD)
    else:
        raise ValueError(eng)


def _emit_synth(ctx, x, dest, w, T, key):
    _emit_synth_phase(ctx, x, dest, w, T, 0, ctx.assign[key + "e"])
    _emit_synth_phase(ctx, x, dest, w, T, 1, ctx.assign[key + "o"])


def _emit_reflect(ctx, xp, L):
    nc = ctx.nc
    nc.vector.tensor_copy(xp[:, 0:7], xp[:, 14:7:-1])
    nc.vector.tensor_copy(xp[:, 7 + L:14 + L], xp[:, L + 5:L - 2:-1])


def build_nc(assign=None):
    a = dict(ASSIGN)
    if assign:
        a.update(assign)
    nc = bacc.Bacc("TRN2", target_bir_lowering=False, debug=False,
                   num_devices=N_CORES)
    x_ap = nc.dram_tensor("x", [ROWS_PER_CORE, L0], F32, kind="ExternalInput").ap()
    y_ap = nc.dram_tensor("y", [4, ROWS_PER_CORE, L0], F32, kind="ExternalOutput").ap()

    with tile.TileContext(nc) as tc:
        with tc.tile_pool(name="bufs", bufs=1) as pool, \
             tc.tile_pool(name="ob", bufs=2) as obpool, \
             tc.tile_pool(name="ps", bufs=8, space="PSUM") as pspool:
            ctx = Ctx(nc, pool, obpool, pspool, a)
            ctx.build_consts()
            # Per-tile overrides: tile0 runs band3 on PE (d1 is ready early,
            # so PE starts ~90us sooner); the last tile runs band0's final
            # stage on DVE (fills DVE's tail idle while PE finishes band1).
            tile_over = [
                {"b3e": PE, "b3o": PE},
                {"b0s3e": V, "b0s3o": V},
            ]
            base_assign = dict(a)

            for t in range(TILES_PER_CORE):
                a = dict(base_assign)
                a.update(tile_over[t % len(tile_over)])
                ctx.assign = a

                # dtype of synthesis-stage inputs: f32r when consumed by PE
                def syn_dt(key):
                    return F32R if (a[key + "e"] == PE or a[key + "o"] == PE) else F32

                rows = slice(t * 128, (t + 1) * 128)

                xp0 = pool.tile([128, L0 + 14], F32, tag="xp0")
                if a["b3e"] == PE:
                    # split the load so d1's first half-chain (which gates
                    # PE's band3 start) only waits for the first half
                    nc.sync.dma_start(xp0[:, 7:7 + 4800], x_ap[rows, 0:4800])
                    nc.sync.dma_start(xp0[:, 7 + 4800:7 + L0], x_ap[rows, 4800:L0])
                else:
                    nc.sync.dma_start(xp0[:, 7:7 + L0], x_ap[rows, :])
                _emit_reflect(ctx, xp0, L0)

                d1 = pool.tile([128, L1], syn_dt("b3"), tag="d1")
                a1p = pool.tile([128, L1 + 14], F32, tag="a1p")
                if a["b3e"] == PE:
                    # d1 gates PE's first work: emit it solo, split into two
                    # half-chains so band3's first PE chunks only wait for
                    # the first half, then a1.
                    Hh = L1 // 2  # 2050: out cols [0, Hh) read xp0[0:2*Hh+6]
                    for k in range(8):
                        src = xp0[:, k:k + 2 * Hh - 1:2]
                        dst = d1[:, 0:Hh]
                        if k == 0:
                            nc.scalar.mul(dst, src, float(DEC_HI[0]))
                        else:
                            nc.vector.scalar_tensor_tensor(
                                dst, src, float(DEC_HI[k]), dst, MULT, ADD)
                    n2 = L1 - Hh
                    for k in range(8):
                        src = xp0[:, 2 * Hh + k:2 * Hh + k + 2 * n2 - 1:2]
                        dst = d1[:, Hh:L1]
                        if k == 0:
                            nc.scalar.mul(dst, src, float(DEC_HI[0]))
                        else:
                            nc.vector.scalar_tensor_tensor(
                                dst, src, float(DEC_HI[k]), dst, MULT, ADD)
                    _emit_ana(ctx, xp0, a1p[:, 7:7 + L1], DEC_LO, L0, a["a1"])
                elif a["d1"] == V and a["a1"] == V:
                    _interleave(_ana_thunks(ctx, xp0, d1[:], DEC_HI, L0),
                                _ana_thunks(ctx, xp0, a1p[:, 7:7 + L1], DEC_LO, L0))
                else:
                    _emit_ana(ctx, xp0, d1[:], DEC_HI, L0, a["d1"])
                    _emit_ana(ctx, xp0, a1p[:, 7:7 + L1], DEC_LO, L0, a["a1"])
                _emit_reflect(ctx, a1p, L1)

                def emit_b3(d1=d1, rows=rows):
                    ob3 = obpool.tile([128, L0], F32, tag="ob")
                    if a["b3e"] == V and a["b3o"] == V:
                        _interleave(
                            _synth_v_thunks(ctx, d1, ob3[:, 0:L0:2],
                                            _synth_phase_taps(REC_HI, 0), L0 // 2),
                            _synth_v_thunks(ctx, d1, ob3[:, 1:L0:2],
                                            _synth_phase_taps(REC_HI, 1), L0 // 2))
                    else:
                        _emit_synth(ctx, d1, ob3, REC_HI, L0, "b3")
                    nc.sync.dma_start(y_ap[3, rows, :], ob3[:])

                if a["b3e"] == PE:
                    emit_b3()  # PE path: start it as early as possible

                d2 = pool.tile([128, L2], syn_dt("b2s1"), tag="d2")
                _emit_ana(ctx, a1p, d2[:], DEC_HI, L1, a["d2"])
                a2p = pool.tile([128, L2 + 14], F32, tag="a2p")
                _emit_ana(ctx, a1p, a2p[:, 7:7 + L2], DEC_LO, L1, a["a2"])
                _emit_reflect(ctx, a2p, L2)

                v_ = pool.tile([128, L1], syn_dt("b2s2"), tag="v")
                _emit_synth(ctx, d2, v_, REC_HI, L1, "b2s1")
                ob2 = obpool.tile([128, L0], F32, tag="ob")
                _emit_synth(ctx, v_, ob2, REC_LO, L0, "b2s2")
                nc.sync.dma_start(y_ap[2, rows, :], ob2[:])

                d3 = pool.tile([128, L3], syn_dt("b1s1"), tag="d3")
                a3 = pool.tile([128, L3], syn_dt("b0s1"), tag="a3")
                if a["d3"] == V and a["a3"] == V:
                    _interleave(_ana_thunks(ctx, a2p, d3[:], DEC_HI, L2),
                                _ana_thunks(ctx, a2p, a3[:], DEC_LO, L2))
                else:
                    _emit_ana(ctx, a2p, d3[:], DEC_HI, L2, a["d3"])
                    _emit_ana(ctx, a2p, a3[:], DEC_LO, L2, a["a3"])

                if a["b3e"] != PE:
                    # DVE band3: emit after all analysis so the PE-gating
                    # deep-level inputs (d2/d3/a3) are produced first.
                    emit_b3()

                u_ = pool.tile([128, L2], syn_dt("b1s2"), tag="u")
                _emit_synth(ctx, d3, u_, REC_HI, L2, "b1s1")
                v_ = pool.tile([128, L1], syn_dt("b1s3"), tag="v")
                _emit_synth(ctx, u_, v_, REC_LO, L1, "b1s2")
                ob1 = obpool.tile([128, L0], F32, tag="ob")
                _emit_synth(ctx, v_, ob1, REC_LO, L0, "b1s3")
                nc.sync.dma_start(y_ap[1, rows, :], ob1[:])

                u_ = pool.tile([128, L2], syn_dt("b0s2"), tag="u")
                _emit_synth(ctx, a3, u_, REC_LO, L2, "b0s1")
                v_ = pool.tile([128, L1], syn_dt("b0s3"), tag="v")
                _emit_synth(ctx, u_, v_, REC_LO, L1, "b0s2")
                ob0 = obpool.tile([128, L0], F32, tag="ob")
                _emit_synth(ctx, v_, ob0, REC_LO, L0, "b0s3")
                nc.sync.dma_start(y_ap[0, rows, :], ob0[:])

    nc.compile()
    return nc


_NC = None


def _get_nc():
    global _NC
    if _NC is None:
        _NC = build_nc()
    return _NC


def shard_inputs(x):
    rows = np.ascontiguousarray(x.reshape(-1, L0))
    return [{"x": rows[c * ROWS_PER_CORE:(c + 1) * ROWS_PER_CORE]}
            for c in range(N_CORES)]


def unshard_outputs(results):
    out = np.empty((4, N_CORES * ROWS_PER_CORE, L0), np.float32)
    for c, r in enumerate(results):
        out[:, c * ROWS_PER_CORE:(c + 1) * ROWS_PER_CORE, :] = r["y"]
    return out.reshape(4, 16, 128, L0)


def kernel(x):
    x = np.asarray(x, np.float32)
    assert x.shape == (16, 128, L0), x.shape
    nc = _get_nc()
    res = run_bass_kernel_spmd(nc, shard_inputs(x), core_ids=list(range(N_CORES)))
    return unshard_outputs(res.results)
